# revision 15
# baseline (speedup 1.0000x reference)
"""CompressedKVAttention Trainium2 Bass kernel (v2).

GQA attention with int8-quantized KV caches, per-(b, kv_head, token)
scale/zero dequant params.  B=4, H=32, HKV=8, QLEN=16, KVLEN=8192, D=128.

Sharding: B x HKV = 32 (batch, kv-head) groups, 4 per core across 8 cores
(data parallel on B, tensor parallel on kv-head groups).  No cross-device
comms.

Per-group math (q' = n_rep*qlen = 64 query rows, t = kv position):
  khat[d, t]  = scale_k[t] * (K[t, d] - zero_k[t]) / sqrt(D)      (host, fp16)
  S[t, q']    = sum_d khat[d, t] * qT[d, q']                      (PE)
  S[t, q']   += ln(scale_v[t])                 (PE rank-8 matmul vs block-id)
  w[t, q']    = exp(S[t, q']) = p[t, q'] * scale_v[t]             (ACT, fp16)
  vf[t, 0:128]= fp16(V_int8[t, d]); vf[t,128] = 1/sv[t]; vf[t,129] = zv[t]
  acc[q', :]  = sum_t w[t, q'] * vf[t, :]                         (PE, accum)
     -> acc[:, 0:128] = sum p sv V ; acc[:,128] = sum p ; acc[:,129] = sum p sv zv
  out[q', d]  = (acc[q', d] - acc[q', 129]) / acc[q', 128]        (DVE)

Scores are computed transposed ([t, q']) so per-token params are per-partition
scalars / matmul columns.  K is pre-transposed+dequantized on the host to fp16
(the PE needs d on partitions; int8 K would need an on-chip transpose that
costs more than the extra DMA).  V stays int8 end-to-end: two slab-wide DVE
copies per group upcast it into the strided [t, 130] rhs layout.
"""

import numpy as np

B, H, HKV, QLEN, KVLEN, D = 4, 32, 8, 16, 8192, 128
NREP = H // HKV            # 4 query heads per kv head
QP = NREP * QLEN           # 64 query rows per group
NCORES = 8
GPC = (B * HKV) // NCORES  # 4 groups per core
TT = 128                   # kv tokens per tile
SLAB = 16                  # score tiles per psum slab (2 banks)
VW = D + 2                 # PV rhs width: 128 v cols + 1/sv + zv

_cached = {}


def _geom(kvlen):
    nt = kvlen // TT
    slab = min(SLAB, nt)
    nslab = nt // slab
    return nt, slab, nslab


def _build_nc(kvlen, gpc, debug=False):
    import concourse.bacc as bacc
    import concourse.tile as tile
    from concourse import mybir

    nt, slab, nslab = _geom(kvlen)
    kh_ch = min(4096, kvlen)   # khat dma chunk width (t)
    vq_ch = min(8192, kvlen)   # v dma chunk width (tokens)
    vt_ch = vq_ch // TT        # tiles per v chunk

    nc = bacc.Bacc("TRN2", target_bir_lowering=False, debug=debug)
    f16, f32, i8 = mybir.dt.float16, mybir.dt.float32, mybir.dt.int8

    khat_d = nc.dram_tensor("khat", [gpc, D, kvlen], f16, kind="ExternalInput")
    vq_d = nc.dram_tensor("vq", [gpc, TT, nt * D], i8, kind="ExternalInput")
    vw = QP + 2 * nt + nslab * TT  # packed vec width: qh | isv | zv | lnsv
    vecs_d = nc.dram_tensor("vecs", [gpc, TT, vw], f16, kind="ExternalInput")
    bconst_d = nc.dram_tensor("bconst", [slab, slab * QP], f16, kind="ExternalInput")
    out_d = nc.dram_tensor("out", [gpc, QP, D], f32, kind="ExternalOutput")

    with tile.TileContext(nc) as tc:
        with (
            tc.tile_pool(name="kh", bufs=8) as kh_pool,
            tc.tile_pool(name="vqp", bufs=4) as vq_pool,
            tc.tile_pool(name="vf", bufs=3) as vf_pool,
            tc.tile_pool(name="w", bufs=6) as w_pool,
            tc.tile_pool(name="vec", bufs=3) as vec_pool,
            tc.tile_pool(name="const", bufs=1) as const_pool,
            tc.tile_pool(name="obuf", bufs=4) as o_pool,
            tc.tile_pool(name="ps", bufs=3, space="PSUM") as ps_pool,
            tc.tile_pool(name="pacc", bufs=2, space="PSUM") as pacc_pool,
        ):
            bconst_t = const_pool.tile([slab, slab * QP], f16)
            nc.sync.dma_start(out=bconst_t[:], in_=bconst_d[:])

            # Software pipelining: PV matmuls for a slab are emitted only
            # after the NEXT slab's QK matmuls, so the PE has score work in
            # flight while ACT computes the exp the PV weights depend on.
            pending = []  # deferred emit closures (PV blocks, epilogues)

            def drain_pending():
                while pending:
                    pending.pop(0)()

            for g in range(gpc):
                # group input loads: first khat chunk 0 (gates the first QK),
                # then the packed small vectors, then the rest of the bulk.
                kh_bounds = [0, 1024] if kvlen > 1024 else [0]
                while kh_bounds[-1] < kvlen:
                    kh_bounds.append(min(kh_bounds[-1] + kh_ch, kvlen))
                kh_chunks = []
                t = kh_pool.tile([D, kh_bounds[1]], f16, tag="kh", name="kh0")
                nc.scalar.dma_start(out=t[:], in_=khat_d[g, :, 0 : kh_bounds[1]])
                kh_chunks.append(t)
                vecs_t = vec_pool.tile([TT, vw], f16, tag="vecs")
                nc.sync.dma_start(out=vecs_t[:], in_=vecs_d[g])
                qh_t = vecs_t[:, 0:QP]
                isv_t = vecs_t[:, QP : QP + nt]
                zv_t = vecs_t[:, QP + nt : QP + 2 * nt]
                lnsv_t = vecs_t[0:slab, QP + 2 * nt : vw]
                vq_chunks = []
                for c in range(kvlen // vq_ch):
                    t = vq_pool.tile([TT, vt_ch * D], i8, tag="vqp")
                    nc.sync.dma_start(
                        out=t[:],
                        in_=vq_d[g, :, c * vt_ch * D : (c + 1) * vt_ch * D],
                    )
                    vq_chunks.append(t)
                for c in range(1, len(kh_bounds) - 1):
                    b0, b1 = kh_bounds[c], kh_bounds[c + 1]
                    t = kh_pool.tile([D, b1 - b0], f16, tag="kh")
                    nc.scalar.dma_start(out=t[:], in_=khat_d[g, :, b0:b1])
                    kh_chunks.append(t)

                # strided V rhs slab: [t_in_tile, tile, 130]
                vf_t = vf_pool.tile([TT, nt, VW], f16, tag="vf")
                for c in range(kvlen // vq_ch):
                    nc.vector.tensor_copy(
                        out=vf_t[:, c * vt_ch : (c + 1) * vt_ch, 0:D],
                        in_=vq_chunks[c][:].rearrange("p (i d) -> p i d", d=D),
                    )
                nc.vector.tensor_copy(out=vf_t[:, :, D : D + 1], in_=isv_t[:, :, None])
                nc.vector.tensor_copy(out=vf_t[:, :, D + 1 : D + 2], in_=zv_t[:, :, None])

                psum_acc = pacc_pool.tile([QP, VW], mybir.dt.float32)

                for s in range(nslab):
                    ps = ps_pool.tile([TT, slab * QP], mybir.dt.float32)
                    for j in range(slab):
                        i = s * slab + j
                        col = i * TT
                        ck = next(n for n in range(len(kh_bounds) - 1)
                                  if kh_bounds[n] <= col < kh_bounds[n + 1])
                        off = col - kh_bounds[ck]
                        nc.tensor.matmul(
                            ps[:, j * QP : (j + 1) * QP],
                            lhsT=kh_chunks[ck][:, off : off + TT],
                            rhs=qh_t[:],
                            start=(j % 8 == 0),  # first MM touching each 2KB zero region
                            stop=False,
                        )
                    # S[t, (j, q')] += ln(sv[t])  via rank-`slab` matmul
                    # (split at 512 moving-operand columns: ISA N limit)
                    nhalf = (slab * QP + 511) // 512
                    for h in range(nhalf):
                        c0, c1 = h * 512, min((h + 1) * 512, slab * QP)
                        nc.tensor.matmul(
                            ps[:, c0:c1],
                            lhsT=lnsv_t[:, s * TT : (s + 1) * TT],
                            rhs=bconst_t[:, c0:c1],
                            start=False,
                            stop=True,  # closes the zero region this half touches
                        )
                    w_t = w_pool.tile([TT, slab * QP], f16, tag="w")
                    nc.scalar.activation(
                        out=w_t[:], in_=ps[:], func=mybir.ActivationFunctionType.Exp
                    )

                    def pv_block(s=s, w_t=w_t, vf_t=vf_t, psum_acc=psum_acc):
                        for j in range(slab):
                            i = s * slab + j
                            nc.tensor.matmul(
                                psum_acc[:],
                                lhsT=w_t[:, j * QP : (j + 1) * QP],
                                rhs=vf_t[:, i, :],
                                start=(i == 0),
                                stop=(i == nt - 1),
                            )

                    if len(pending) >= 2:
                        pending.pop(0)()
                    pending.append(pv_block)

                def epilogue(g=g, psum_acc=psum_acc):
                    # out = (acc[:, 0:D] - acc[:, D+1]) * (1 / acc[:, D])
                    rec_t = o_pool.tile([QP, 1], f32, tag="rec")
                    nc.vector.reciprocal(out=rec_t[:], in_=psum_acc[:, D : D + 1])
                    c_t = o_pool.tile([QP, 1], f32, tag="c")
                    nc.vector.tensor_copy(out=c_t[:], in_=psum_acc[:, D + 1 : D + 2])
                    o_t = o_pool.tile([QP, D], f32, tag="o")
                    nc.vector.tensor_scalar(
                        out=o_t[:],
                        in0=psum_acc[:, 0:D],
                        scalar1=c_t[:],
                        scalar2=rec_t[:],
                        op0=mybir.AluOpType.subtract,
                        op1=mybir.AluOpType.mult,
                    )
                    nc.gpsimd.dma_start(out=out_d[g], in_=o_t[:])

                pending.append(epilogue)
            drain_pending()

    nc.compile()
    return nc


def _host_prep(query, key_cache, value_cache, key_scale, key_zero,
               value_scale, value_zero, kvlen=KVLEN, ncores=NCORES, gpc=GPC):
    """Build per-core input maps. Groups are (b, kv_head) pairs, flat index
    b*HKV + kvh, gpc consecutive groups per core."""
    nt, slab, nslab = _geom(kvlen)
    scale = 1.0 / np.sqrt(D)
    bconst = np.repeat(np.eye(slab, dtype=np.float16), QP, axis=1)
    vw = QP + 2 * nt + nslab * TT
    in_maps = []
    for c in range(ncores):
        khat = np.empty((gpc, D, kvlen), np.float16)
        vqp = np.empty((gpc, TT, nt * D), np.int8)
        vecs = np.zeros((gpc, TT, vw), np.float16)
        qhat = vecs[:, :, 0:QP]
        isvp = vecs[:, :, QP : QP + nt]
        zvp = vecs[:, :, QP + nt : QP + 2 * nt]
        lnsv = vecs[:, 0:slab, QP + 2 * nt : vw]
        for g in range(gpc):
            flat = c * gpc + g
            b, kvh = divmod(flat, HKV)
            k = key_cache[b, kvh].astype(np.float32)          # [t, d]
            kz = key_zero[b, kvh][:, None]
            ks = key_scale[b, kvh][:, None]
            khat[g] = ((k - kz) * (ks * scale)).T.astype(np.float16)
            q = query[b, kvh * NREP : (kvh + 1) * NREP]        # [nrep, qlen, d]
            qhat[g] = q.reshape(QP, D).T.astype(np.float16)
            # v p-major: vqp[p, i*D + d] = V[i*TT + p, d]
            vqp[g] = (
                value_cache[b, kvh]
                .reshape(nt, TT, D)
                .transpose(1, 0, 2)
                .reshape(TT, nt * D)
            )
            sv = value_scale[b, kvh].astype(np.float32)
            isvp[g] = (1.0 / sv).reshape(nt, TT).T.astype(np.float16)
            zvp[g] = value_zero[b, kvh].reshape(nt, TT).T.astype(np.float16)
            # lnsv[j, s*TT + p] = ln(sv[s*slab*TT + j*TT + p])
            # centered at ln(0.01): the constant scales num/den/c accumulators
            # uniformly and cancels in the final ratio; centering keeps the
            # fp16 rounding error of the correction ~3e-4.
            lnsv[g] = (
                (np.log(sv) - np.log(0.01))
                .reshape(nslab, slab, TT).transpose(1, 0, 2).reshape(slab, nslab * TT)
            ).astype(np.float16)
        in_maps.append({"khat": khat, "vq": vqp, "vecs": vecs, "bconst": bconst})
    return in_maps


def _host_post(results, ncores=NCORES, gpc=GPC):
    out = np.empty((B, H, QLEN, D), np.float32)
    for c in range(ncores):
        o = results[c]["out"]  # [gpc, QP, D]
        for g in range(gpc):
            flat = c * gpc + g
            b, kvh = divmod(flat, HKV)
            out[b, kvh * NREP : (kvh + 1) * NREP] = o[g].reshape(NREP, QLEN, D)
    return out


def kernel(query, key_cache, value_cache, key_scale, key_zero,
           value_scale, value_zero):
    from concourse.bass_utils import run_bass_kernel_spmd

    if "nc" not in _cached:
        _cached["nc"] = _build_nc(KVLEN, GPC)
    nc = _cached["nc"]
    in_maps = _host_prep(
        np.asarray(query), np.asarray(key_cache), np.asarray(value_cache),
        np.asarray(key_scale), np.asarray(key_zero),
        np.asarray(value_scale), np.asarray(value_zero),
    )
    res = run_bass_kernel_spmd(nc, in_maps, core_ids=list(range(NCORES)))
    return _host_post(res.results)


# revision 16
# speedup vs baseline: 1.1202x; 1.1202x over previous
"""CompressedKVAttention Trainium2 Bass kernel (v2).

GQA attention with int8-quantized KV caches, per-(b, kv_head, token)
scale/zero dequant params.  B=4, H=32, HKV=8, QLEN=16, KVLEN=8192, D=128.

Sharding: B x HKV = 32 (batch, kv-head) groups, 4 per core across 8 cores
(data parallel on B, tensor parallel on kv-head groups).  No cross-device
comms.

Per-group math (q' = n_rep*qlen = 64 query rows, t = kv position):
  khat[d, t]  = scale_k[t] * (K[t, d] - zero_k[t]) / sqrt(D)      (host, fp16)
  S[t, q']    = sum_d khat[d, t] * qT[d, q']                      (PE)
  S[t, q']   += ln(scale_v[t])                 (PE rank-8 matmul vs block-id)
  w[t, q']    = exp(S[t, q']) = p[t, q'] * scale_v[t]             (ACT, fp16)
  vf[t, 0:128]= fp16(V_int8[t, d]); vf[t,128] = 1/sv[t]; vf[t,129] = zv[t]
  acc[q', :]  = sum_t w[t, q'] * vf[t, :]                         (PE, accum)
     -> acc[:, 0:128] = sum p sv V ; acc[:,128] = sum p ; acc[:,129] = sum p sv zv
  out[q', d]  = (acc[q', d] - acc[q', 129]) / acc[q', 128]        (DVE)

Scores are computed transposed ([t, q']) so per-token params are per-partition
scalars / matmul columns.  K is pre-transposed+dequantized on the host to fp16
(the PE needs d on partitions; int8 K would need an on-chip transpose that
costs more than the extra DMA).  V stays int8 end-to-end: two slab-wide DVE
copies per group upcast it into the strided [t, 130] rhs layout.
"""

import numpy as np

B, H, HKV, QLEN, KVLEN, D = 4, 32, 8, 16, 8192, 128
NREP = H // HKV            # 4 query heads per kv head
QP = NREP * QLEN           # 64 query rows per group
NCORES = 8
GPC = (B * HKV) // NCORES  # 4 groups per core
TT = 128                   # kv tokens per tile
SLAB = 16                  # score tiles per psum slab (2 banks)
VW = D + 2                 # PV rhs width: 128 v cols + 1/sv + zv

_cached = {}


def _geom(kvlen):
    nt = kvlen // TT
    slab = min(SLAB, nt)
    nslab = nt // slab
    return nt, slab, nslab


def _build_nc(kvlen, gpc, debug=False):
    import concourse.bacc as bacc
    import concourse.tile as tile
    from concourse import mybir

    nt, slab, nslab = _geom(kvlen)
    kh_ch = min(4096, kvlen)   # khat dma chunk width (t)
    vq_ch = min(8192, kvlen)   # v dma chunk width (tokens)
    vt_ch = vq_ch // TT        # tiles per v chunk

    nc = bacc.Bacc("TRN2", target_bir_lowering=False, debug=debug)
    f16, f32, i8 = mybir.dt.float16, mybir.dt.float32, mybir.dt.int8

    khat_d = nc.dram_tensor("khat", [gpc, D, kvlen], f16, kind="ExternalInput")
    vq_d = nc.dram_tensor("vq", [gpc, TT, nt * D], i8, kind="ExternalInput")
    vw = QP + 2 * nt + nslab * TT  # packed vec width: qh | isv | zv | lnsv
    vecs_d = nc.dram_tensor("vecs", [gpc, TT, vw], f16, kind="ExternalInput")
    bconst_d = nc.dram_tensor("bconst", [slab, slab * QP], f16, kind="ExternalInput")
    out_d = nc.dram_tensor("out", [gpc, QP, D], f32, kind="ExternalOutput")

    with tile.TileContext(nc) as tc:
        with (
            tc.tile_pool(name="kh", bufs=8) as kh_pool,
            tc.tile_pool(name="vqp", bufs=4) as vq_pool,
            tc.tile_pool(name="vf", bufs=3) as vf_pool,
            tc.tile_pool(name="w", bufs=6) as w_pool,
            tc.tile_pool(name="vec", bufs=3) as vec_pool,
            tc.tile_pool(name="const", bufs=1) as const_pool,
            tc.tile_pool(name="obuf", bufs=4) as o_pool,
            tc.tile_pool(name="ps", bufs=3, space="PSUM") as ps_pool,
            tc.tile_pool(name="pacc", bufs=2, space="PSUM") as pacc_pool,
        ):
            bconst_t = const_pool.tile([slab, slab * QP], f16)
            nc.sync.dma_start(out=bconst_t[:], in_=bconst_d[:])

            # Software pipelining: PV matmuls for a slab are emitted only
            # after the NEXT slab's QK matmuls, so the PE has score work in
            # flight while ACT computes the exp the PV weights depend on.
            pending = []  # deferred emit closures (PV blocks, epilogues)

            def drain_pending():
                while pending:
                    pending.pop(0)()

            for g in range(gpc):
                # group input loads: first khat chunk 0 (gates the first QK),
                # then the packed small vectors, then the rest of the bulk.
                kh_bounds = [0, 1024] if kvlen > 1024 else [0]
                while kh_bounds[-1] < kvlen:
                    kh_bounds.append(min(kh_bounds[-1] + kh_ch, kvlen))
                kh_chunks = []
                t = kh_pool.tile([D, kh_bounds[1]], f16, tag="kh", name="kh0")
                nc.sync.dma_start(out=t[:], in_=khat_d[g, :, 0 : kh_bounds[1]])
                kh_chunks.append(t)
                vecs_t = vec_pool.tile([TT, vw], f16, tag="vecs")
                nc.sync.dma_start(out=vecs_t[:], in_=vecs_d[g])
                qh_t = vecs_t[:, 0:QP]
                isv_t = vecs_t[:, QP : QP + nt]
                zv_t = vecs_t[:, QP + nt : QP + 2 * nt]
                lnsv_t = vecs_t[0:slab, QP + 2 * nt : vw]
                vq_chunks = []
                for c in range(kvlen // vq_ch):
                    t = vq_pool.tile([TT, vt_ch * D], i8, tag="vqp")
                    nc.sync.dma_start(
                        out=t[:],
                        in_=vq_d[g, :, c * vt_ch * D : (c + 1) * vt_ch * D],
                    )
                    vq_chunks.append(t)
                for c in range(1, len(kh_bounds) - 1):
                    b0, b1 = kh_bounds[c], kh_bounds[c + 1]
                    t = kh_pool.tile([D, b1 - b0], f16, tag="kh")
                    nc.sync.dma_start(out=t[:], in_=khat_d[g, :, b0:b1])
                    kh_chunks.append(t)

                # strided V rhs slab: [t_in_tile, tile, 130]
                vf_t = vf_pool.tile([TT, nt, VW], f16, tag="vf")
                for c in range(kvlen // vq_ch):
                    nc.vector.tensor_copy(
                        out=vf_t[:, c * vt_ch : (c + 1) * vt_ch, 0:D],
                        in_=vq_chunks[c][:].rearrange("p (i d) -> p i d", d=D),
                    )
                nc.vector.tensor_copy(out=vf_t[:, :, D : D + 1], in_=isv_t[:, :, None])
                nc.vector.tensor_copy(out=vf_t[:, :, D + 1 : D + 2], in_=zv_t[:, :, None])

                psum_acc = pacc_pool.tile([QP, VW], mybir.dt.float32)

                for s in range(nslab):
                    ps = ps_pool.tile([TT, slab * QP], mybir.dt.float32)
                    for j in range(slab):
                        i = s * slab + j
                        col = i * TT
                        ck = next(n for n in range(len(kh_bounds) - 1)
                                  if kh_bounds[n] <= col < kh_bounds[n + 1])
                        off = col - kh_bounds[ck]
                        nc.tensor.matmul(
                            ps[:, j * QP : (j + 1) * QP],
                            lhsT=kh_chunks[ck][:, off : off + TT],
                            rhs=qh_t[:],
                            start=(j % 8 == 0),  # first MM touching each 2KB zero region
                            stop=False,
                        )
                    # S[t, (j, q')] += ln(sv[t])  via rank-`slab` matmul
                    # (split at 512 moving-operand columns: ISA N limit)
                    nhalf = (slab * QP + 511) // 512
                    for h in range(nhalf):
                        c0, c1 = h * 512, min((h + 1) * 512, slab * QP)
                        nc.tensor.matmul(
                            ps[:, c0:c1],
                            lhsT=lnsv_t[:, s * TT : (s + 1) * TT],
                            rhs=bconst_t[:, c0:c1],
                            start=False,
                            stop=True,  # closes the zero region this half touches
                        )
                    w_t = w_pool.tile([TT, slab * QP], f16, tag="w")
                    nc.scalar.activation(
                        out=w_t[:], in_=ps[:], func=mybir.ActivationFunctionType.Exp
                    )

                    def pv_block(s=s, w_t=w_t, vf_t=vf_t, psum_acc=psum_acc):
                        for j in range(slab):
                            i = s * slab + j
                            nc.tensor.matmul(
                                psum_acc[:],
                                lhsT=w_t[:, j * QP : (j + 1) * QP],
                                rhs=vf_t[:, i, :],
                                start=(i == 0),
                                stop=(i == nt - 1),
                            )

                    if len(pending) >= 2:
                        pending.pop(0)()
                    pending.append(pv_block)

                def epilogue(g=g, psum_acc=psum_acc):
                    # out = (acc[:, 0:D] - acc[:, D+1]) * (1 / acc[:, D])
                    rec_t = o_pool.tile([QP, 1], f32, tag="rec")
                    nc.vector.reciprocal(out=rec_t[:], in_=psum_acc[:, D : D + 1])
                    c_t = o_pool.tile([QP, 1], f32, tag="c")
                    nc.vector.tensor_copy(out=c_t[:], in_=psum_acc[:, D + 1 : D + 2])
                    o_t = o_pool.tile([QP, D], f32, tag="o")
                    nc.vector.tensor_scalar(
                        out=o_t[:],
                        in0=psum_acc[:, 0:D],
                        scalar1=c_t[:],
                        scalar2=rec_t[:],
                        op0=mybir.AluOpType.subtract,
                        op1=mybir.AluOpType.mult,
                    )
                    nc.gpsimd.dma_start(out=out_d[g], in_=o_t[:])

                pending.append(epilogue)
            drain_pending()

    nc.compile()
    return nc


def _host_prep(query, key_cache, value_cache, key_scale, key_zero,
               value_scale, value_zero, kvlen=KVLEN, ncores=NCORES, gpc=GPC):
    """Build per-core input maps. Groups are (b, kv_head) pairs, flat index
    b*HKV + kvh, gpc consecutive groups per core."""
    nt, slab, nslab = _geom(kvlen)
    scale = 1.0 / np.sqrt(D)
    bconst = np.repeat(np.eye(slab, dtype=np.float16), QP, axis=1)
    vw = QP + 2 * nt + nslab * TT
    in_maps = []
    for c in range(ncores):
        khat = np.empty((gpc, D, kvlen), np.float16)
        vqp = np.empty((gpc, TT, nt * D), np.int8)
        vecs = np.zeros((gpc, TT, vw), np.float16)
        qhat = vecs[:, :, 0:QP]
        isvp = vecs[:, :, QP : QP + nt]
        zvp = vecs[:, :, QP + nt : QP + 2 * nt]
        lnsv = vecs[:, 0:slab, QP + 2 * nt : vw]
        for g in range(gpc):
            flat = c * gpc + g
            b, kvh = divmod(flat, HKV)
            k = key_cache[b, kvh].astype(np.float32)          # [t, d]
            kz = key_zero[b, kvh][:, None]
            ks = key_scale[b, kvh][:, None]
            khat[g] = ((k - kz) * (ks * scale)).T.astype(np.float16)
            q = query[b, kvh * NREP : (kvh + 1) * NREP]        # [nrep, qlen, d]
            qhat[g] = q.reshape(QP, D).T.astype(np.float16)
            # v p-major: vqp[p, i*D + d] = V[i*TT + p, d]
            vqp[g] = (
                value_cache[b, kvh]
                .reshape(nt, TT, D)
                .transpose(1, 0, 2)
                .reshape(TT, nt * D)
            )
            sv = value_scale[b, kvh].astype(np.float32)
            isvp[g] = (1.0 / sv).reshape(nt, TT).T.astype(np.float16)
            zvp[g] = value_zero[b, kvh].reshape(nt, TT).T.astype(np.float16)
            # lnsv[j, s*TT + p] = ln(sv[s*slab*TT + j*TT + p])
            # centered at ln(0.01): the constant scales num/den/c accumulators
            # uniformly and cancels in the final ratio; centering keeps the
            # fp16 rounding error of the correction ~3e-4.
            lnsv[g] = (
                (np.log(sv) - np.log(0.01))
                .reshape(nslab, slab, TT).transpose(1, 0, 2).reshape(slab, nslab * TT)
            ).astype(np.float16)
        in_maps.append({"khat": khat, "vq": vqp, "vecs": vecs, "bconst": bconst})
    return in_maps


def _host_post(results, ncores=NCORES, gpc=GPC):
    out = np.empty((B, H, QLEN, D), np.float32)
    for c in range(ncores):
        o = results[c]["out"]  # [gpc, QP, D]
        for g in range(gpc):
            flat = c * gpc + g
            b, kvh = divmod(flat, HKV)
            out[b, kvh * NREP : (kvh + 1) * NREP] = o[g].reshape(NREP, QLEN, D)
    return out


def kernel(query, key_cache, value_cache, key_scale, key_zero,
           value_scale, value_zero):
    from concourse.bass_utils import run_bass_kernel_spmd

    if "nc" not in _cached:
        _cached["nc"] = _build_nc(KVLEN, GPC)
    nc = _cached["nc"]
    in_maps = _host_prep(
        np.asarray(query), np.asarray(key_cache), np.asarray(value_cache),
        np.asarray(key_scale), np.asarray(key_zero),
        np.asarray(value_scale), np.asarray(value_zero),
    )
    res = run_bass_kernel_spmd(nc, in_maps, core_ids=list(range(NCORES)))
    return _host_post(res.results)
